# revision 19
# baseline (speedup 1.0000x reference)
"""Trainium2 Bass kernel for nn_EncoderSimilarity (block-cosine similarity).

sims[a,b] = sum over block-granularities {128, 256} of
            sum_t max_v ( l2norm(img_block_v) . l2norm(cap_block_t) )

Sharding: img rows (axis a) split 8 ways across cores, cap replicated;
each core computes its [256, 2048] slice of sims.

v2 device algorithm (per core), built around three measured HW facts:
  (1) engine writes into PSUM survive matmul(start=False) accumulation,
  (2) VectorE reduce_max can span multiple PSUM banks with a strided AP,
  (3) fp32-PSUM reads run at 1 elem/cycle on both VectorE and ScalarE,
      so PSUM drain traffic must be minimized and split across engines.

Max-of-8 restructure relative to a base block (v*=7 odd):
  max_v L[v,t] = L[7,t] + excess[t]
  excess = max(m0,m1,m2,m3),  m_p = Delta[2p+1] + relu(D_p)  (p=0..2)
                              m_3 = relu(D_3)
  where D_p = L[2p]-L[2p+1], Delta[v] = L[v]-L[7] come straight from
  matmuls with differenced img weights.  ScalarE relu's D in-place in
  PSUM; the Delta matmul accumulates on top (no vector add at all).
  sum_t L[7,t] factors through the PE: one matmul against capsum
  (itself accumulated by identity matmuls).  The t-sum of excess slabs
  also runs on the PE as identity-matmul accumulation into one fp32
  PSUM bank, so VectorE only does: one 2-bank reduce_max + one
  PSUM/SBUF max + one bf16 max per t-pair.
"""
import sys

if "/opt/trn_rl_repo" not in sys.path:
    sys.path.insert(0, "/opt/trn_rl_repo")

from contextlib import ExitStack

import numpy as np

N_CORES = 8
A, B, C = 2048, 2048, 1024
A_PER = A // N_CORES          # 256 img rows per core
NQ = 4                        # b processed in quarters of 512
BQ = B // NQ                  # 512


def _build_kernel():
    import concourse.bass as bass
    import concourse.tile as tile
    from concourse import mybir

    F32 = mybir.dt.float32
    BF16 = mybir.dt.bfloat16
    Alu = mybir.AluOpType
    Act = mybir.ActivationFunctionType
    Ax = mybir.AxisListType

    nc = bass.Bass(
        trn_type="TRN2",
        target_bir_lowering=False,
        debug=False,
        num_devices=N_CORES,
    )
    img_d = nc.dram_tensor("img", [A_PER, C], F32, kind="ExternalInput").ap()
    cap_d = nc.dram_tensor("cap", [B, C], F32, kind="ExternalInput").ap()
    ident_d = nc.dram_tensor("ident", [128, 128], BF16, kind="ExternalInput").ap()
    out_d = nc.dram_tensor("sims", [A_PER, B], F32, kind="ExternalOutput").ap()

    with tile.TileContext(nc) as tc, ExitStack() as ctx:
        _body(ctx, tc, out_d, img_d, cap_d, ident_d, F32, BF16, Alu, Act, Ax)
    return nc


def _body(ctx, tc, out_d, img_d, cap_d, ident_d, F32, BF16, Alu, Act, Ax):
    nc = tc.nc

    dram = ctx.enter_context(tc.tile_pool(name="dram", bufs=1, space="DRAM"))
    persist = ctx.enter_context(tc.tile_pool(name="persist", bufs=1))
    norm = ctx.enter_context(tc.tile_pool(name="norm", bufs=2))
    small = ctx.enter_context(tc.tile_pool(name="small", bufs=3))
    stage = ctx.enter_context(tc.tile_pool(name="stage", bufs=2))
    drain = ctx.enter_context(tc.tile_pool(name="drain", bufs=2))
    prep = ctx.enter_context(tc.tile_pool(name="prep", bufs=4))
    capin = ctx.enter_context(tc.tile_pool(name="capin", bufs=5))
    psum = ctx.enter_context(tc.tile_pool(name="psum", bufs=4, space="PSUM"))

    ident = persist.tile([128, 128], BF16, tag="ident")
    nc.sync.dma_start(ident[:], ident_d[:])

    # ---------------- normalization helper (natural [n, c] layout) -------------
    def normalize_tile(x_f32, n128_out, n256_out, nm, gp=True, pool=None):
        """x_f32 [128, 1024] fp32 -> block-l2-normalized bf16 tiles.

        Steady state (gp=True): squares + scale-muls on GPSIMD (otherwise
        idle).  Startup path (gp=False, img + quarter 0): Scalar/Vector,
        which have lower latency before the main loop saturates them.
        """
        sq = (pool or norm).tile([128, C], F32, tag="sq", name=f"sq_{nm}", bufs=3)
        if gp:
            nc.gpsimd.tensor_mul(sq[:], x_f32[:], x_f32[:])
        else:
            nc.scalar.activation(sq[:], x_f32[:], Act.Square)
        s12 = small.tile([128, 12], F32, tag="s12", name=f"s12_{nm}")
        nc.vector.reduce_sum(
            s12[:, 0:8], sq.rearrange("p (j c) -> p j c", c=128), axis=Ax.X
        )
        nc.vector.tensor_tensor(
            s12[:, 8:12],
            s12.rearrange("p (k two) -> p k two", two=2)[:, 0:4, 0],
            s12.rearrange("p (k two) -> p k two", two=2)[:, 0:4, 1],
            op=Alu.add,
        )
        rt = small.tile([128, 12], F32, tag="rt", name=f"rt_{nm}")
        nc.scalar.activation(rt[:], s12[:], Act.Sqrt)
        inv = small.tile([128, 12], F32, tag="inv", name=f"inv_{nm}")
        nc.vector.reciprocal(inv[:], rt[:])
        eng = nc.gpsimd if gp else nc.vector
        eng.tensor_mul(
            n128_out.rearrange("p (j c) -> p j c", c=128),
            x_f32.rearrange("p (j c) -> p j c", c=128),
            inv[:, 0:8].unsqueeze(2).to_broadcast((128, 8, 128)),
        )
        eng.tensor_mul(
            n256_out.rearrange("p (k c) -> p k c", c=256),
            x_f32.rearrange("p (k c) -> p k c", c=256),
            inv[:, 8:12].unsqueeze(2).to_broadcast((128, 4, 256)),
        )

    # ---------------- img prep -> transposed bf16 weight tiles -----------------
    # w128T slots: 0..3 = D_p = n128[2p]-n128[2p+1]; 4..6 = wd_j = n128[2j+1]-
    # n128[7]; 7 = base n128[7].
    # w256T slots: 0..3 = D'_i half h (2i+h); 4..5 = wd' h; 6..7 = base' h.
    w128T = [persist.tile([128, 8, 128], BF16, tag=f"w128T_{at}", name=f"w128T_{at}") for at in range(2)]
    w256T = [persist.tile([128, 8, 128], BF16, tag=f"w256T_{at}", name=f"w256T_{at}") for at in range(2)]
    for at in range(2):
        x = norm.tile([128, C], F32, tag="img_in", name=f"img_in_{at}")
        nc.sync.dma_start(x[:], img_d[at * 128:(at + 1) * 128, :])
        n128 = norm.tile([128, C], BF16, tag="img_n128", name=f"img_n128_{at}")
        n256 = norm.tile([128, C], BF16, tag="img_n256", name=f"img_n256_{at}")
        normalize_tile(x, n128, n256, f"img{at}", gp=False)

        d128 = norm.tile([128, 8, 128], BF16, tag="d128", name=f"d128_{at}")
        v128 = n128.rearrange("p (v c) -> p v c", c=128)
        nc.vector.tensor_tensor(d128[:, 0:4, :], v128[:, 0::2, :], v128[:, 1::2, :],
                                op=Alu.subtract)
        nc.vector.tensor_tensor(
            d128[:, 4:7, :], v128[:, 1:7:2, :],
            v128[:, 7:8, :].to_broadcast((128, 3, 128)), op=Alu.subtract)
        nc.vector.tensor_copy(d128[:, 7, :], v128[:, 7, :])

        d256 = norm.tile([128, 8, 128], BF16, tag="d256", name=f"d256_{at}")
        v256 = n256.rearrange("p (v c) -> p v c", c=256)
        nc.vector.tensor_tensor(
            d256.rearrange("p (i h) c -> p i (h c)", h=2)[:, 0:2, :],
            v256[:, 0::2, :], v256[:, 1::2, :], op=Alu.subtract)
        nc.vector.tensor_tensor(d256[:, 4:6, :].rearrange("p h c -> p (h c)"),
                                v256[:, 1, :], v256[:, 3, :], op=Alu.subtract)
        nc.vector.tensor_copy(d256[:, 6:8, :].rearrange("p h c -> p (h c)"),
                              v256[:, 3, :])

        for gi, (src, dstT) in enumerate(((d128, w128T[at]), (d256, w256T[at]))):
            for jg in range(2):
                pt = psum.tile([128, 4, 128], BF16, tag="u2",
                               name=f"ptw_{at}_{gi}_{jg}")
                for k in range(4):
                    nc.tensor.transpose(pt[:, k, :], src[:, jg * 4 + k, :], ident[:])
                if (gi + jg) % 2 == 0:
                    nc.vector.tensor_copy(dstT[:, jg * 4:(jg + 1) * 4, :], pt[:])
                else:
                    nc.scalar.copy(dstT[:, jg * 4:(jg + 1) * 4, :], pt[:])

    # ---------------- cap prep (per quarter) -----------------------------------
    scr_c128 = dram.tile([B, C], BF16, tag="scr_c128")
    scr_c256 = dram.tile([B, C], BF16, tag="scr_c256")
    scr_cs128 = dram.tile([B, 128], BF16, tag="scr_cs128")
    scr_cs256 = dram.tile([B, 256], BF16, tag="scr_cs256")

    def prep_quarter(q):
        c128q = persist.tile([128, 8, BQ], BF16, tag=f"capT128_{q % 3}", name=f"capT128_{q}")
        c256q = persist.tile([128, 8, BQ], BF16, tag=f"capT256_{q % 3}", name=f"capT256_{q}")
        cs0stage = (persist.tile([128, 3, 128 * 4], BF16, tag="cs0stage",
                                 name="cs0stage") if q == 0 else None)
        for r in range(4):  # row-tiles within quarter
            row0 = q * BQ + r * 128
            x = capin.tile([128, C], F32, tag="cap_in", name=f"cap_in_{q}_{r}")
            nc.sync.dma_start(x[:], cap_d[row0:row0 + 128, :])
            n128 = prep.tile([128, C], BF16, tag="cap_n128", name=f"cap_n128_{q}_{r}")
            n256 = prep.tile([128, C], BF16, tag="cap_n256", name=f"cap_n256_{q}_{r}")
            normalize_tile(x, n128, n256, f"cap{q}_{r}", gp=(q > 0), pool=prep)
            # pre-transpose capsum folds (tiny; q>0 rides the DMA-transpose
            # path, q==0 goes through PE transposes like the cap tiles)
            v8 = n128.rearrange("p (t c) -> p t c", c=128)
            pf1 = small.tile([128, 4, 128], BF16, tag="pf1", bufs=2,
                             name=f"pf1_{q}_{r}")
            nc.vector.tensor_tensor(pf1[:], v8[:, 0:4], v8[:, 4:8],
                                    op=Alu.add)
            pf2 = small.tile([128, 2, 128], BF16, tag="pf2", bufs=2,
                             name=f"pf2_{q}_{r}")
            nc.vector.tensor_tensor(pf2[:], pf1[:, 0:2], pf1[:, 2:4],
                                    op=Alu.add)
            pcs1 = small.tile([128, 128], BF16, tag="pcs1", bufs=2,
                              name=f"pcs1_{q}_{r}")
            nc.vector.tensor_tensor(pcs1[:], pf2[:, 0], pf2[:, 1],
                                    op=Alu.add)
            w4 = n256.rearrange("p (t c) -> p t c", c=256)
            pg1 = small.tile([128, 2, 256], BF16, tag="pg1", bufs=2,
                             name=f"pg1_{q}_{r}")
            nc.vector.tensor_tensor(pg1[:], w4[:, 0:2], w4[:, 2:4],
                                    op=Alu.add)
            pcs2 = small.tile([128, 256], BF16, tag="pcs2", bufs=2,
                              name=f"pcs2_{q}_{r}")
            nc.vector.tensor_tensor(pcs2[:], pg1[:, 0], pg1[:, 1],
                                    op=Alu.add)
            if q == 0:
                # PE transposes straight from SBUF: no DRAM roundtrip latency
                for half, (srcT, dstq) in enumerate(((n128, c128q), (n256, c256q))):
                    for jg in range(2):
                        pt = psum.tile([128, 4, 128], BF16, tag="u2",
                                       name=f"pt_{q}_{r}_{half}_{jg}")
                        for k in range(4):
                            j = jg * 4 + k
                            nc.tensor.transpose(
                                pt[:, k, :], srcT[:, j * 128:(j + 1) * 128], ident[:]
                            )
                        dst = dstq[:, jg * 4:(jg + 1) * 4, r * 128:(r + 1) * 128]
                        if (half + jg) % 2 == 0:
                            nc.vector.tensor_copy(dst, pt[:])
                        else:
                            nc.scalar.copy(dst, pt[:])
                ptc = psum.tile([128, 3, 128], BF16, tag="u2",
                                name=f"ptc_{q}_{r}")
                nc.tensor.transpose(ptc[:, 0, :], pcs1[:], ident[:])
                for h in range(2):
                    nc.tensor.transpose(ptc[:, 1 + h, :],
                                        pcs2[:, h * 128:(h + 1) * 128], ident[:])
                nc.vector.tensor_copy(
                    cs0stage[:, :, r * 128:(r + 1) * 128], ptc[:])
            else:
                nc.sync.dma_start(scr_c128[row0:row0 + 128, :], n128[:])
                nc.sync.dma_start(scr_c256[row0:row0 + 128, :], n256[:])
                nc.sync.dma_start(scr_cs128[row0:row0 + 128, :], pcs1[:])
                nc.sync.dma_start(scr_cs256[row0:row0 + 128, :], pcs2[:])
        cs128 = persist.tile([128, BQ], BF16, tag=f"cs128_{q % 3}", name=f"cs128_{q}")
        cs256 = persist.tile([128, 2, BQ], BF16, tag=f"cs256_{q % 3}", name=f"cs256_{q}")
        if q > 0:
            for j in range(8):
                nc.sync.dma_start_transpose(
                    c128q[:, j, :], scr_c128[q * BQ:(q + 1) * BQ, j * 128:(j + 1) * 128]
                )
                nc.sync.dma_start_transpose(
                    c256q[:, j, :], scr_c256[q * BQ:(q + 1) * BQ, j * 128:(j + 1) * 128]
                )
            nc.sync.dma_start_transpose(
                cs128[:], scr_cs128[q * BQ:(q + 1) * BQ, :])
            for h in range(2):
                nc.sync.dma_start_transpose(
                    cs256[:, h, :],
                    scr_cs256[q * BQ:(q + 1) * BQ, h * 128:(h + 1) * 128])
        else:
            nc.vector.tensor_copy(cs128[:], cs0stage[:, 0, :])
            nc.scalar.copy(cs256[:], cs0stage[:, 1:3, :])
        return c128q, c256q, cs128, cs256

    # ---------------- main loop ------------------------------------------------
    def main_quarter(q, c128q, c256q, cs128, cs256, mid_emit=None):
        def do_tq(at, tq, stg):
            # pD matmuls for all four pair-units stream first; seed relus
            # follow (r3's relu is emitted after the seeds -- it is needed
            # one V-op later); Delta matmuls accumulate onto the seeds.
            p3 = psum.tile([128, 2, BQ], F32, tag="u2",
                           name=f"p3_{q}_{at}_{tq}")
            units = [psum.tile([128, 2, BQ], F32, tag="u2",
                               name=f"u_{q}_{at}_{tq}_{p}")
                     for p in range(3)]
            for ti in range(2):
                nc.tensor.matmul(p3[:, ti, :], w128T[at][:, 3, :],
                                 c128q[:, 2 * tq + ti, :],
                                 start=True, stop=True)
            for p in range(3):
                for ti in range(2):
                    nc.tensor.matmul(units[p][:, ti, :], w128T[at][:, p, :],
                                     c128q[:, 2 * tq + ti, :],
                                     start=True, stop=True)
            nc.scalar.activation(units[0][:], units[0][:], Act.Relu)
            r3 = drain.tile([128, 2, BQ], BF16, tag="r3",
                            name=f"r3_{q}_{at}_{tq}")
            nc.scalar.activation(r3[:], p3[:], Act.Relu)
            nc.scalar.activation(units[1][:], units[1][:], Act.Relu)
            nc.scalar.activation(units[2][:], units[2][:], Act.Relu)
            for p in range(3):
                for ti in range(2):
                    nc.tensor.matmul(units[p][:, ti, :], w128T[at][:, 4 + p, :],
                                     c128q[:, 2 * tq + ti, :],
                                     start=False, stop=True,
                                     skip_group_check=True)
            xm = drain.tile([128, 2, BQ], BF16, tag="xm",
                            name=f"xm_{q}_{at}_{tq}")
            nc.vector.tensor_tensor(xm[:], units[0][:], r3[:], op=Alu.max)
            ym = drain.tile([128, 2, BQ], BF16, tag="ym",
                            name=f"ym_{q}_{at}_{tq}")
            nc.vector.tensor_tensor(ym[:], units[1][:], xm[:], op=Alu.max)
            nc.vector.tensor_tensor(stg[:, tq], units[2][:], ym[:],
                                    op=Alu.max)

        def do_tqp(at, tqp, stg):
            pc1 = psum.tile([128, 2, BQ], F32, tag="u2",
                            name=f"pc1_{q}_{at}_{tqp}")
            for tpi in range(2):
                for h in range(2):
                    nc.tensor.matmul(
                        pc1[:, tpi, :], w256T[at][:, 2 + h, :],
                        c256q[:, 2 * (2 * tqp + tpi) + h, :],
                        start=(h == 0), stop=(h == 1))
            pc0 = psum.tile([128, 2, BQ], F32, tag="u2",
                            name=f"pc0_{q}_{at}_{tqp}")
            for tpi in range(2):
                for h in range(2):
                    nc.tensor.matmul(
                        pc0[:, tpi, :], w256T[at][:, h, :],
                        c256q[:, 2 * (2 * tqp + tpi) + h, :],
                        start=(h == 0), stop=(h == 1))
            nc.scalar.activation(pc0[:], pc0[:], Act.Relu)
            r1 = drain.tile([128, 2, BQ], BF16, tag="r1",
                            name=f"r1_{q}_{at}_{tqp}")
            nc.scalar.activation(r1[:], pc1[:], Act.Relu)
            for tpi in range(2):
                for h in range(2):
                    nc.tensor.matmul(
                        pc0[:, tpi, :], w256T[at][:, 4 + h, :],
                        c256q[:, 2 * (2 * tqp + tpi) + h, :],
                        start=False, stop=(h == 1),
                        skip_group_check=True)
            nc.vector.tensor_tensor(stg[:, 4 + tqp], pc0[:], r1[:],
                                    op=Alu.max)

        for at in range(2):
            if at == 1 and mid_emit is not None:
                mid_emit()
            asl = slice(at * 128, (at + 1) * 128)
            stg = stage.tile([128, 6, 2, BQ], BF16, tag="stg",
                             name=f"stg_{q}_{at}")
            # interleaved schedule smooths the Scalar/Vector mix
            do_tq(at, 0, stg)
            do_tq(at, 1, stg)
            do_tqp(at, 0, stg)
            do_tq(at, 2, stg)
            do_tq(at, 3, stg)
            do_tqp(at, 1, stg)
            # ---- t-sum: one Vector fold (6 slabs -> 3), then PE-accumulate
            # the 3 slabs + base terms into one fp32 PSUM bank ----
            nc.vector.tensor_tensor(stg[:, 0:3], stg[:, 0:3], stg[:, 3:6],
                                    op=Alu.add)
            acc = psum.tile([128, BQ], F32, tag="u2", name=f"acc_{q}_{at}")
            nc.tensor.matmul(acc[:], w128T[at][:, 7, :], cs128[:],
                             start=True, stop=False)
            for h in range(2):
                nc.tensor.matmul(acc[:], w256T[at][:, 6 + h, :], cs256[:, h, :],
                                 start=False, stop=False, skip_group_check=True)
            for s in range(3):
                for ti in range(2):
                    nc.tensor.matmul(acc[:], ident[:], stg[:, s, ti, :],
                                     start=False, stop=(s == 2 and ti == 1),
                                     skip_group_check=True)
            accs = drain.tile([128, BQ], F32, tag="accs", name=f"accs_{q}_{at}")
            nc.scalar.copy(accs[:], acc[:])
            nc.sync.dma_start(out_d[asl, q * BQ:(q + 1) * BQ], accs[:])

    caps = {0: prep_quarter(0), 1: prep_quarter(1)}
    for q in range(NQ):
        def mid_emit(q=q):
            if q + 2 < NQ:
                caps[q + 2] = prep_quarter(q + 2)
        main_quarter(q, *caps[q], mid_emit=mid_emit)
        del caps[q]


_NC_CACHE = None


# ---------------------------------------------------------------------------
# Workaround: this container's walrus build rejects instructions with more
# than one sync-wait condition ("Too many sync wait commands").  Split the
# extra waits onto sequencer-only RegisterMove carrier instructions in a BIR
# post-pass, and monkeypatch the compile entry points to apply it.
import json as _json


def _split_multiwaits(bir_bytes: bytes) -> bytes:
    m = _json.loads(bir_bytes)
    uid = [0]

    def carrier(engine, wait, debug):
        uid[0] += 1
        return {
            "debug": debug,
            "engine": engine,
            "ins": [{"dtype": "int32", "kind": "imm_value", "value": 0}],
            "outs": [{"dtype": "int32", "kind": "register_access",
                      "regref": f"{engine}_zero"}],
            "name": f"I-wsplit-{uid[0]}",
            "opcode": "RegisterMove",
            "sync_info": {"on_update": [], "on_wait": [wait]},
        }

    for f in m["functions"]:
        for bb in f["blocks"]:
            out = []
            for inst in bb["instructions"]:
                si = inst.get("sync_info")
                waits = (si or {}).get("on_wait") or []
                eng = inst.get("engine")
                if len(waits) > 1 and eng and eng != "Unassigned":
                    for w in waits[:-1]:
                        out.append(carrier(eng, w, inst.get("debug", 0)))
                    si["on_wait"] = [waits[-1]]
                out.append(inst)
            bb["instructions"] = out
    return _json.dumps(m).encode()


def _install_birpatch():
    import concourse.bass_utils as bu
    import concourse.bass2jax as b2j

    if getattr(bu.compile_bir_kernel, "_wsplit_wrapped", False):
        return
    orig = bu.compile_bir_kernel

    def wrapped(bir_json: bytes, tmpdir: str, neff_name="file.neff"):
        return orig(_split_multiwaits(bir_json), tmpdir, neff_name=neff_name)

    wrapped._wsplit_wrapped = True
    bu.compile_bir_kernel = wrapped
    b2j.compile_bir_kernel = wrapped


def kernel(img_emb: np.ndarray, cap_emb: np.ndarray) -> np.ndarray:
    _install_birpatch()
    from concourse.bass_utils import run_bass_kernel_spmd

    global _NC_CACHE
    if _NC_CACHE is None:
        _NC_CACHE = _build_kernel()
    nc = _NC_CACHE

    import ml_dtypes

    img = np.ascontiguousarray(np.asarray(img_emb, dtype=np.float32))
    cap = np.ascontiguousarray(np.asarray(cap_emb, dtype=np.float32))
    ident = np.eye(128, dtype=ml_dtypes.bfloat16)
    in_maps = [
        {"img": img[k * A_PER:(k + 1) * A_PER], "cap": cap, "ident": ident}
        for k in range(N_CORES)
    ]
    res = run_bass_kernel_spmd(nc, in_maps, core_ids=list(range(N_CORES)))
    return np.concatenate([r["sims"] for r in res.results], axis=0)


if __name__ == "__main__":
    rng = np.random.default_rng(0)
    img = rng.normal(size=(A, C)).astype(np.float32)
    cap = rng.normal(size=(B, C)).astype(np.float32)
    out = kernel(img, cap)
    print("out", out.shape, out.dtype, float(out.min()), float(out.max()))
